# revision 48
# baseline (speedup 1.0000x reference)
"""CFAR OS-CA 2D detector kernel for Trainium2 (8 NeuronCores, Bass/Tile).

Algorithm
---------
reference: per (batch, vel) row of 1024 range cells (circular):
  OS stage: miu[r] = 8th largest of 32 training cells at r +- [5..20];
            os = alpha * miu
  CA stage: out[v] = mean over vel offsets +-[3..10] (circular) of os

Kernel strategy (per core = 2 batches = 512 rows, range on the free axis,
bf16 selection pipeline, ~3.4e-3 max rel err vs the fp32 reference).
Scheduling notes vs the earlier revision: the per-depth prefix/suffix scans
are fused into one double-width scan over [F pages | junction pad | mirrored
pages] (tiles >0; tile 0 keeps two alternating half-scans so its Pool pad
memsets hide behind the other chain); mscan is double-buffered to break the
cross-tile WAR handoff; the activation table is pre-warmed; halo DMAs are
replaced by wrap-reads during the cast; alpha/16 is folded into the CA
weights; each (half, chunk) CA accumulation gets its own PSUM bank and the
last tile tapers its final chunks (512/256/128/128) with per-chunk os tiles
so the drain is short.
  * van Herk / Gil-Werman on 16-blocks: for every block, the sorted top-8 of
    every prefix (and, scanning backward, every suffix) is built with a chain
    of 8 tensor_tensor_scan recurrences
        m_k[c] = min(max(x[c], state), m_{k-1}[c -/+ 1])
    which is provably the k-th-largest prefix recurrence; a single -1e30 pad
    column per block (17-column pages, re-cleaned by tiny gpsimd stripe
    memsets after each chain) both resets the state at block boundaries and
    serves as the "empty prefix" table entry.
  * every 16-wide window = one block suffix + next-block prefix; the top-8
    multiset of two sorted-desc 8-lists A,B is {max(A_i, B_{7-i})}
    ("valley"); a 3-stage bitonic merge sorts it descending -> W8(s) table.
    Valley/resort/final are emitted as multi-plane batched TT ops (bf16 ->
    2x DVE mode).
  * OS output: 8th largest of union of the two 16-windows at r-20 and r+5 =
    min_i max(W8(r-20)_i, W8(r+5)_{7-i}).
  * CA stage: circulant matmul on the tensor engine ([vel,vel] banded 0/1
    bf16 weights, fp32 PSUM accumulate; alpha/16 applied in fp32 on the
    ScalarE PSUM eviction) -- no transpose needed since vel sits on
    partitions.
Batch is pure data parallel across the 8 cores (no halo exchange needed).
Engines: DVE does all selection math (~190us busy); ScalarE casts/evicts,
GpSimd does pad-stripe memsets, PE does the CA matmul, all overlapped.
"""

import sys

if "/opt/trn_rl_repo" not in sys.path:
    sys.path.insert(0, "/opt/trn_rl_repo")

import math
from contextlib import ExitStack

import numpy as np

import concourse.mybir as mybir
from concourse import bacc, bass_utils
from concourse.ap import AP
from concourse.tile import TileContext

F32 = mybir.dt.float32
MIN = mybir.AluOpType.min
MAX = mybir.AluOpType.max
NEG = -1.0e30

# ---- module hyperparameters (match the nn.Module) ----
G = (2, 4)
T = (8, 16)
PFA = 1e-05
K_ORDER = 24
OS_N = 2 * T[1]          # 32
HR = G[1] + T[1]         # 20
HV = G[0] + T[0]         # 10


def _os_cfar_threshold(k, n, pfa):
    def log_factorial(n):
        n = n + 1
        if n < 9:
            return np.log(math.factorial(n))
        return 1 / 2 * (np.log(2 * np.pi) - np.log(n)) + n * (
            np.log(n + 1 / (12 * n - 1 / 10 / n)) - 1
        )

    def fun(k, n, t_os, pfa):
        return (
            log_factorial(n)
            - log_factorial(n - k)
            - np.sum(np.log(np.arange(n, n - k, -1) + t_os))
            - np.log(pfa)
        )

    t_max, t_min = 1e32, 1.0
    for _ in range(10000):
        m_n = t_max - fun(k, n, t_max, pfa) * (t_min - t_max) / (
            fun(k, n, t_min, pfa) - fun(k, n, t_max, pfa)
        )
        f_m_n = fun(k, n, m_n, pfa)
        if f_m_n == 0 or np.abs(t_max - t_min) < 0.0001:
            return m_n
        if fun(k, n, t_max, pfa) * f_m_n < 0:
            t_min = m_n
        elif fun(k, n, t_min, pfa) * f_m_n < 0:
            t_max = m_n
        else:
            break
    raise ValueError("CFAR threshold did not converge.")


OS_ALPHA = float(np.sqrt(_os_cfar_threshold(K_ORDER, OS_N, PFA)))

# ---- problem/shard geometry ----
B, V, R = 16, 256, 1024
NCORES = 8
BPC = B // NCORES        # batches per core
ROWS = BPC * V           # 512 rows per core
NT = ROWS // 128         # 4 partition tiles
HALO = 32
XC = R + 2 * HALO        # 1088 haloed columns
NBLK = XC // 16          # 68 16-blocks
# page layout: [p0, y0..y15] per block; a single pad column per block is
# kept clean (-1e30) by a tiny gpsimd stripe-memset after each scan chain,
# which both resets the next chain at block starts and serves as the
# empty-prefix entry for the window merge.
PADS = 1
PGW = PADS + 16          # 17
W1H = NBLK * PGW         # 1156
# fused scan layout: [F pages (68) | junction pad page boundary | mirrored
# pages (68)] -> one forward scan computes both the prefix chain (F section)
# and the suffix chain (R section = column-mirrored data, so a forward scan
# over it is a backward scan over the original positions).
WTOT = (2 * NBLK + 1) * PGW  # 2329 (137 pages; data cols end at 2312)
SCE = WTOT - PGW             # 2312: scan range is [PGW, SCE)
VB = 67                  # window-table blocks
VW = VB * 16             # 1072 window-start columns (s = col - 32)
BF16 = mybir.dt.bfloat16


def _ca_weights() -> np.ndarray:
    # Mfull[vi, vo] = alpha/16 where (vi - vo) mod 256 in {3..10, 246..253}
    # (scale folded into the weights so the PSUM result can be DMAed out
    # directly; bf16 weight quantization is ~0.4% rel, well inside the 2e-2
    # gate)
    import ml_dtypes

    d = np.arange(128)[:, None] - np.arange(128)[None, :]

    def f(dm):
        dm = np.mod(dm, 256)
        return ((dm >= 3) & (dm <= 10)) | ((dm >= 246) & (dm <= 253))

    scale = np.float32(OS_ALPHA / (2 * T[0]))
    w_diag = f(d).astype(np.float32) * scale
    w_cross = f(d + 128).astype(np.float32) * scale
    return np.ascontiguousarray(
        np.stack([w_diag, w_cross]).astype(ml_dtypes.bfloat16)
    )


def build_kernel():
    nc = bacc.Bacc(
        "TRN2",
        target_bir_lowering=False,
        debug=False,
        enable_asserts=False,
        num_devices=NCORES,
    )
    data = nc.dram_tensor("data", [ROWS, R], F32, kind="ExternalInput").ap()
    caw = nc.dram_tensor("caw", [2, 128, 128], BF16, kind="ExternalInput").ap()
    out = nc.dram_tensor("out", [ROWS, R], F32, kind="ExternalOutput").ap()

    COPY = mybir.ActivationFunctionType.Copy

    with TileContext(nc) as tc, ExitStack() as ctx:
        cpool = ctx.enter_context(tc.tile_pool(name="const", bufs=1))
        iopool = ctx.enter_context(tc.tile_pool(name="io", bufs=4))
        wpool = ctx.enter_context(tc.tile_pool(name="work", bufs=1))
        ospool = ctx.enter_context(tc.tile_pool(name="os", bufs=1))
        ppool = ctx.enter_context(tc.tile_pool(name="psum", bufs=8, space="PSUM"))
        opool = ctx.enter_context(tc.tile_pool(name="outb", bufs=2))

        # warm the activation table at t=0 so the 1.3us LoadActFuncSet is off
        # the first cast's critical path
        warm = cpool.tile([128, 1], BF16)
        nc.gpsimd.memset(warm[:], 0.0)
        nc.scalar.activation(out=warm[:], in_=warm[:], func=COPY)

        # constants: min-gate plane for the first scan (+BIG, -BIG at pads)
        gate = cpool.tile([128, WTOT], BF16)
        nc.vector.memset(gate[:], 1e30)
        gate3 = gate.rearrange("p (m c) -> p m c", c=PGW)
        nc.vector.memset(gate3[:, :, 0:PADS], NEG)
        # CA circulant weight blocks [vi, vo] (0/1 in bf16); loaded after
        # tile 0's data DMA so they don't delay the first scan chain
        w_sb = cpool.tile([128, 256], BF16)

        os_tiles = {}
        for t in range(NT):
            rows = slice(128 * t, 128 * t + 128)
            # ---- load (no halo DMAs: the circular halo blocks are read
            # back out of the main region during the cast) ----
            xc = iopool.tile([128, R], F32, tag="xc")
            if t == 0:
                # tile 0's load gates the whole pipeline: split across the
                # two HWDGE queues so the first cast piece starts early
                nc.sync.dma_start(out=xc[:, 0:512], in_=data[rows, 0:512])
                nc.scalar.dma_start(out=xc[:, 512:R], in_=data[rows, 512:R])
                nc.gpsimd.dma_start(out=w_sb[:, 0:128], in_=caw[0])
                nc.gpsimd.dma_start(out=w_sb[:, 128:256], in_=caw[1])
            else:
                nc.sync.dma_start(out=xc[:], in_=data[rows, :])
            xc3 = xc.rearrange("p (b c) -> p b c", c=16)

            # ---- padded page layout [F pages | jpad | mirrored pages] ----
            # R[2311 - 17b - j] = xpad[16b + j]: a forward scan over R is a
            # backward scan over the original positions (per-block suffixes).
            # xpr block b holds x[(16b - 32) mod 1024 ...]: blocks 0-1 wrap
            # to x-blocks 62-63, blocks 66-67 wrap to x-blocks 0-1.
            xpr2 = wpool.tile([128, WTOT], BF16, tag=f"xpr{t % 2}", name="xpr")
            x2v = xpr2.rearrange("p (m c) -> p m c", c=PGW)
            nc.gpsimd.memset(x2v[:, :, 0:PADS], NEG)
            cast_pieces = (
                ((2, 34, 0), (66, 68, 0), (34, 66, 32), (0, 2, 62))
                if t == 0 else
                ((2, 66, 0), (0, 2, 62), (66, 68, 0))
            )
            for b0, b1, i0 in cast_pieces:
                nc.scalar.activation(out=x2v[:, b0:b1, PADS:PGW],
                                     in_=xc3[:, i0 : i0 + (b1 - b0)],
                                     func=COPY)
            # mirrored copy (fp32 -> bf16): Pool normally; tile 0 uses the
            # idle DVE so the fill isn't gated on Pool's slower copy
            mir_eng = nc.vector if t == 0 else nc.gpsimd
            for b0, b1, i0 in cast_pieces:
                rout = AP(xpr2.tensor, xpr2.offset + 2311 - 17 * b0,
                          [list(xpr2.ap[0]), [-17, b1 - b0], [-1, 16]])
                mir_eng.tensor_copy(out=rout, in_=xc3[:, i0 : i0 + (b1 - b0)])

            # ---- fused prefix+suffix sorted top-8 scan chains ----
            # m_k[c] = min(max(x[c], state), m_{k-1}[c - 1]); the clean pad
            # column in m_{k-1} resets the state at each block boundary (and
            # at the F|R junction). One scan per depth covers both chains.
            mscan = wpool.tile([128, 8 * WTOT], BF16, tag=f"mscan{t % 2}")
            mk = [mscan[:, k * WTOT : (k + 1) * WTOT] for k in range(8)]
            if t == 0:
                # tile 0 has no prior-tile merge work for the scheduler to
                # slot between fused scans, so its Pool pad-memsets would
                # stall every scan; run the two chains as separate
                # alternating scans instead (each chain's memset hides
                # behind the other chain's scan)
                for k in range(8):
                    mpk = mk[k].rearrange("p (m c) -> p m c", c=PGW)
                    if k == 0:
                        nc.vector.tensor_tensor_scan(
                            out=mk[0][:, PGW:W1H], data0=gate[:, PGW:W1H],
                            data1=xpr2[:, PGW:W1H],
                            initial=NEG, op0=MIN, op1=MAX,
                        )
                    else:
                        nc.vector.tensor_tensor_scan(
                            out=mk[k][:, PGW:W1H], data0=xpr2[:, PGW:W1H],
                            data1=mk[k - 1][:, PGW - 1 : W1H - 1],
                            initial=NEG, op0=MAX, op1=MIN,
                        )
                    nc.gpsimd.memset(mpk[:, 0:NBLK, 0:PADS], NEG)
                    if k == 0:
                        nc.vector.tensor_tensor_scan(
                            out=mk[0][:, W1H + PADS : SCE],
                            data0=gate[:, W1H + PADS : SCE],
                            data1=xpr2[:, W1H + PADS : SCE],
                            initial=NEG, op0=MIN, op1=MAX,
                        )
                    else:
                        nc.vector.tensor_tensor_scan(
                            out=mk[k][:, W1H + PADS : SCE],
                            data0=xpr2[:, W1H + PADS : SCE],
                            data1=mk[k - 1][:, W1H : SCE - 1],
                            initial=NEG, op0=MAX, op1=MIN,
                        )
                    nc.gpsimd.memset(mpk[:, NBLK : 2 * NBLK + 1, 0:PADS], NEG)
            else:
                nc.vector.tensor_tensor_scan(
                    out=mk[0][:, PGW:SCE], data0=gate[:, PGW:SCE],
                    data1=xpr2[:, PGW:SCE], initial=NEG, op0=MIN, op1=MAX,
                )
                mp0 = mk[0].rearrange("p (m c) -> p m c", c=PGW)
                nc.gpsimd.memset(mp0[:, :, 0:PADS], NEG)
                for k in range(1, 8):
                    nc.vector.tensor_tensor_scan(
                        out=mk[k][:, PGW:SCE], data0=xpr2[:, PGW:SCE],
                        data1=mk[k - 1][:, PGW - 1 : SCE - 1],
                        initial=NEG, op0=MAX, op1=MIN,
                    )
                    mpk = mk[k].rearrange("p (m c) -> p m c", c=PGW)
                    nc.gpsimd.memset(mpk[:, :, 0:PADS], NEG)

            # ---- valley planes: top-8 multiset of each 16-window ----
            # va[i] = max( suffix view (R section of mk[i]),
            #              prefix view (F section of mk[7-i]) )
            # suffix of block b from j -> R col 2311 - 17b - j
            # prefix of block b+1 len j -> F col 17b + 17 + j (j=0 -> pad)
            va = wpool.tile([128, 8 * VW], BF16, tag="va")

            vb = wpool.tile([128, 8 * VW], BF16, tag="vb")
            suf = AP(mscan.tensor, mscan.offset + 2311,
                     [list(mscan.ap[0]), [WTOT, 8], [-17, VB], [-1, 16]])
            pre = AP(mscan.tensor, mscan.offset + 7 * WTOT + PGW,
                     [list(mscan.ap[0]), [-WTOT, 8], [17, VB], [1, 16]])
            dst = va.rearrange("p (s b c) -> p s b c", b=VB, c=16)
            nc.vector.tensor_tensor(out=dst[:], in0=suf, in1=pre, op=MAX)

            def planes(buf, plist, width=VW, off=0):
                # uniform-stride plane list, or 2x2 block structure (e.g. 0,1,4,5)
                base = plist[0]
                if len(plist) == 4 and plist[1] - plist[0] != plist[3] - plist[2]:
                    raise AssertionError(plist)
                if len(plist) == 4 and plist[2] - plist[0] != plist[1] - plist[0] * 0 + (
                    plist[1] - plist[0]
                ) * 2:
                    inner = plist[1] - plist[0]
                    outer = plist[2] - plist[0]
                    return AP(buf.tensor, buf.offset + base * VW + off,
                              [list(buf.ap[0]), [outer * VW, 2], [inner * VW, 2], [1, width]])
                step = plist[1] - plist[0] if len(plist) > 1 else 1
                return AP(buf.tensor, buf.offset + base * VW + off,
                          [list(buf.ap[0]), [step * VW, len(plist)], [1, width]])

            # ---- bitonic resort of the valley (descending), batched ----
            RW, RO = 1050, 12     # consumed W8-table columns: [12, 1062)
            for srcs, dsts, lo, hi in (
                (va, vb, (0, 1, 2, 3), (4, 5, 6, 7)),
                (vb, va, (0, 1, 4, 5), (2, 3, 6, 7)),
                (va, vb, (0, 2, 4, 6), (1, 3, 5, 7)),
            ):
                nc.vector.tensor_tensor(
                    out=planes(dsts, lo, width=RW, off=RO),
                    in0=planes(srcs, lo, width=RW, off=RO),
                    in1=planes(srcs, hi, width=RW, off=RO), op=MAX)
                nc.vector.tensor_tensor(
                    out=planes(dsts, hi, width=RW, off=RO),
                    in0=planes(srcs, lo, width=RW, off=RO),
                    in1=planes(srcs, hi, width=RW, off=RO), op=MIN)

            # ---- final OS merge of the two windows ----
            # F_i = max(T_i[col r+12], T_{7-i}[col r+37]); then min-tree
            if t % 2 == 0:
                ca_ps = {}
            else:
                ca_outp = {h: opool.tile([128, R], F32, tag=f"outp{h}", name=f"outp{h}")
                           for h in (0, 1)}
            os_t = ospool.tile([128, R], BF16, tag=f"os{t}", name=f"os{t}")
            # full-width final for tiles 0..NT-2; tapered chunks on the last
            # tile so the drain (matmul->evict->DMA of the last chunk) is
            # short. alpha/16 lives in the CA weights so the eviction is a
            # plain copy; stores are consolidated per 512-half so the SP/ACT
            # DMA queues don't thrash during the drain.
            os_chunks = {}

            def _emit_ca(co, cw):
                # each (half, chunk) accumulates in its OWN psum bank so an
                # eviction only waits for its own pair of matmuls
                cols = slice(co, co + cw)
                if t == NT - 1:
                    os_rhs = os_chunks[co][:, 0:cw]
                else:
                    os_rhs = os_t[:, cols]
                for half in (0, 1):
                    if t % 2 == 0:
                        w_first = w_sb[:, 0:128] if half == 0 else w_sb[:, 128:256]
                        ps = ppool.tile([128, 512], F32, tag="ps",
                                        name=f"ps{half}{co}")
                        nc.tensor.matmul(
                            out=ps[:, 0:cw], lhsT=w_first, rhs=os_rhs,
                            start=True, stop=False,
                        )
                        ca_ps[(half, co)] = ps
                    else:
                        w_second = w_sb[:, 128:256] if half == 0 else w_sb[:, 0:128]
                        ps = ca_ps[(half, co)]
                        nc.tensor.matmul(
                            out=ps[:, 0:cw], lhsT=w_second, rhs=os_rhs,
                            start=False, stop=True,
                        )
                        outp = ca_outp[half]
                        if t == NT - 1 and co == 896:
                            # DVE is idle during the drain: evict there so the
                            # scalar queue is free for the last h1 store
                            nc.vector.tensor_copy(out=outp[:, cols],
                                                  in_=ps[:, 0:cw])
                        else:
                            nc.scalar.activation(
                                out=outp[:, cols], in_=ps[:, 0:cw], func=COPY,
                            )
                        orows = slice(128 * (t - 1 + half), 128 * (t + half))
                        end = co + cw
                        # two stores per half on the SP queue: a DMA on the
                        # scalar queue head-of-line blocks evictions behind it
                        if end == 512:
                            nc.sync.dma_start(out=out[orows, 0:512],
                                              in_=outp[:, 0:512])
                        elif end == R:
                            eng = nc.sync
                            if t == NT - 1 and half == 1:
                                eng = nc.scalar
                            eng.dma_start(out=out[orows, 512:R],
                                          in_=outp[:, 512:R])

            CHUNKS = ((0, 512), (512, 256), (768, 128), (896, 128))
            fchunks = ((0, R),) if t < NT - 1 else CHUNKS
            for co, cw in fchunks:
                rev8 = AP(vb.tensor, vb.offset + 7 * VW + 37 + co,
                          [list(vb.ap[0]), [-VW, 8], [1, cw]])
                fw8 = AP(vb.tensor, vb.offset + 12 + co,
                         [list(vb.ap[0]), [VW, 8], [1, cw]])
                fdst = AP(va.tensor, va.offset + co, [list(va.ap[0]), [VW, 8], [1, cw]])
                nc.vector.tensor_tensor(out=fdst, in0=fw8, in1=rev8, op=MAX)
                nc.vector.tensor_tensor(
                    out=planes(va, (0, 1, 2, 3), width=cw, off=co),
                    in0=planes(va, (0, 1, 2, 3), width=cw, off=co),
                    in1=planes(va, (4, 5, 6, 7), width=cw, off=co), op=MIN)
                nc.vector.tensor_tensor(
                    out=planes(va, (0, 1), width=cw, off=co),
                    in0=planes(va, (0, 1), width=cw, off=co),
                    in1=planes(va, (2, 3), width=cw, off=co), op=MIN)
                if t == NT - 1:
                    # separate per-chunk os tiles: a shared os tile makes the
                    # last chunk's write WAR-wait on earlier chunks' matmuls
                    os_c = ospool.tile([128, cw], BF16, tag=f"osc{co}",
                                       name=f"osc{co}")
                    os_chunks[co] = os_c
                    os_dst = os_c[:, 0:cw]
                else:
                    os_dst = os_t[:, co : co + cw]
                nc.vector.tensor_tensor(
                    out=os_dst, in0=planes(va, (0,), width=cw, off=co),
                    in1=planes(va, (1,), width=cw, off=co), op=MIN)
                if t == NT - 1:
                    # last tile: emit this chunk's CA right away (tail overlap)
                    _emit_ca(co, cw)
            if t < NT - 1:
                for cco, ccw in CHUNKS:
                    _emit_ca(cco, ccw)
            os_tiles[t] = os_t

    nc.compile()
    return nc


_NC_CACHE = None


def _get_nc():
    global _NC_CACHE
    if _NC_CACHE is None:
        _NC_CACHE = build_kernel()
    return _NC_CACHE


def run(data: np.ndarray, trace: bool = False, trace_kwargs=None):
    data = np.ascontiguousarray(np.asarray(data, dtype=np.float32))
    assert data.shape == (B, V, R), data.shape
    nc = _get_nc()
    caw = _ca_weights()
    in_maps = [
        {"data": np.ascontiguousarray(data[BPC * c : BPC * (c + 1)].reshape(ROWS, R)),
         "caw": caw}
        for c in range(NCORES)
    ]
    try:
        res = bass_utils.run_bass_kernel_spmd(
            nc, in_maps, core_ids=list(range(NCORES)),
            trace=trace, **(trace_kwargs or {}),
        )
    except ModuleNotFoundError:
        # no NTFF hook in this environment -- run without tracing
        res = bass_utils.run_bass_kernel_spmd(
            nc, in_maps, core_ids=list(range(NCORES)), trace=False,
        )
    outs = [res.results[c]["out"].reshape(BPC, V, R) for c in range(NCORES)]
    return np.concatenate(outs, axis=0), res


def kernel(data: np.ndarray) -> np.ndarray:
    out, _ = run(data)
    return out

